# revision 3
# baseline (speedup 1.0000x reference)
"""GCN (2-layer GraphConv) Trainium2 Bass kernel, 8-core SPMD.

Strategy (dst-sharded graph parallel):
- Nodes partitioned into 8 shards of 6250 (core c owns dst nodes [6250c, 6250(c+1))).
- Edges assigned to the core owning their dst; split by src half (int16 gather idx).
- Transform tables replicated: every core computes h = (x @ W1) * d_out for ALL
  nodes into a local DRAM table (rows shifted +1; rows 0 and 50001 are zero, used
  as the gather target for padding tokens).
- Aggregation per core: per (dst, half) group, edges are padded to quads of 4
  (pads gather the zero row). Groups sorted by unit count m = ceil(deg/4)
  descending, packed 768 groups per chunk. Gathered chunk layout
  [T0 | T2 | T1 | T3] where T0 = [U_1 | ... | U_M] slabs (U_u = u-th quad of each
  group with m >= u, 128-rounded caps K_u uniform across cores per chunk index).
  Two slab adds reduce quads (t0+=t1/t2+=t3, then +=), then chain slab adds fold
  U_u into U_1. Each group ends as ONE token; dma_scatter_add writes it to row
  half*6400+dst — all rows unique per call and across calls (the instruction
  loses updates on duplicate rows - verified on HW).
- Layer 2: p2 = (relu(sum_halves(agg)*d_in + b1) @ W2pad) * d_out on the owned
  shard, AllGathered into table2 (same +1 row shift), then the same chunk
  structure aggregates 64-float tokens with the same index arrays.
- Output: each core returns its [6250, 32] shard; the host concatenates.
"""

import numpy as np

import concourse.bacc as bacc
import concourse.bass as bass  # noqa: F401
import concourse.mybir as mybir
import concourse.tile as tile
from concourse import bass_utils

N_NODES = 50000
N_CORES = 8
SHARD = 6250
HALF_N = 25000
F_IN = 128
HID = 128
NCLS = 32
NCLS_PAD = 64

NT_X = 391  # ceil(50000/128) node tiles for the h-table build
NPAD = NT_X * 128  # 50048
NT_S = 49  # ceil(6250/128) shard tiles

G_CAP = 768  # groups per chunk (scatter tokens per chunk)
ZERO_IDX_0 = 0  # zero row idx for half 0 (table row 0)
ZERO_IDX_1 = 25000  # zero row idx for half 1 (table row 50001 - base 25001)
AGG_H = 6400  # agg rows per half
TRASH0 = 2 * AGG_H  # trash rows base
AGG_ROWS = TRASH0 + 768

_DT = mybir.dt.float32


def _build_program(struct):
    """struct[h] = list of chunks; chunk = list of K_u (each a multiple of 128).
    Builds + compiles the 8-core SPMD program."""
    nc = bacc.Bacc("TRN2", target_bir_lowering=False, debug=False,
                   num_devices=N_CORES)

    tot_gtok = sum(4 * sum(ch) for h in range(2) for ch in struct[h])
    tot_stok = sum(G_CAP for h in range(2) for _ in struct[h])

    xT = nc.dram_tensor("xT", [128, NPAD], _DT, kind="ExternalInput")
    W1 = nc.dram_tensor("W1", [F_IN, HID], _DT, kind="ExternalInput")
    W2p = nc.dram_tensor("W2p", [HID, NCLS_PAD], _DT, kind="ExternalInput")
    b1bc = nc.dram_tensor("b1bc", [128, HID], _DT, kind="ExternalInput")
    b2bc = nc.dram_tensor("b2bc", [128, NCLS_PAD], _DT, kind="ExternalInput")
    ident = nc.dram_tensor("ident", [128, 128], _DT, kind="ExternalInput")
    dout_pm = nc.dram_tensor("dout_pm", [128, NT_X], _DT, kind="ExternalInput")
    dinsh = nc.dram_tensor("dinsh", [128, NT_S], _DT, kind="ExternalInput")
    doutsh = nc.dram_tensor("doutsh", [128, NT_S], _DT, kind="ExternalInput")
    gidx = nc.dram_tensor("gidx", [128, tot_gtok // 16], mybir.dt.int16,
                          kind="ExternalInput")
    sidx = nc.dram_tensor("sidx", [128, tot_stok // 16], mybir.dt.int16,
                          kind="ExternalInput")
    out = nc.dram_tensor("out", [SHARD, NCLS], _DT, kind="ExternalOutput")

    # +1 row shift tables: row 0 and row 50001 are zeros
    table1 = nc.dram_tensor("table1", [NPAD + 2, HID], _DT, kind="Internal")
    agg1 = nc.dram_tensor("agg1", [AGG_ROWS, HID], _DT, kind="Internal")
    agg2 = nc.dram_tensor("agg2", [AGG_ROWS, NCLS_PAD], _DT, kind="Internal")
    p2b = nc.dram_tensor("p2b", [SHARD, NCLS_PAD], _DT, kind="Internal")
    table2 = nc.dram_tensor("table2", [N_NODES + 2, NCLS_PAD], _DT,
                            kind="Internal", addr_space="Shared")

    t1ap = table1.ap()
    t2ap = table2.ap()

    with tile.TileContext(nc) as tc:
        with (
            tc.tile_pool(name="const", bufs=1) as cpool,
            tc.tile_pool(name="xload", bufs=3) as xpool,
            tc.tile_pool(name="hstore", bufs=3) as hpool,
            tc.tile_pool(name="mmps", bufs=2, space="PSUM") as mmps,
            tc.tile_pool(name="idx", bufs=4) as ipool,
            tc.tile_pool(name="buf1", bufs=2) as bpool1,
            tc.tile_pool(name="buf2", bufs=2) as bpool2,
            tc.tile_pool(name="post", bufs=3) as ppool,
            tc.tile_pool(name="pps", bufs=2, space="PSUM") as pps,
            tc.tile_pool(name="zero", bufs=1) as zpool,
        ):
            # ---- constants ----
            w1_s = cpool.tile([F_IN, HID], _DT)
            nc.sync.dma_start(w1_s[:], W1.ap())
            w2_s = cpool.tile([HID, NCLS_PAD], _DT)
            nc.sync.dma_start(w2_s[:], W2p.ap())
            b1_s = cpool.tile([128, HID], _DT)
            nc.sync.dma_start(b1_s[:], b1bc.ap())
            b2_s = cpool.tile([128, NCLS_PAD], _DT)
            nc.sync.dma_start(b2_s[:], b2bc.ap())
            id_s = cpool.tile([128, 128], _DT)
            nc.sync.dma_start(id_s[:], ident.ap())
            do_s = cpool.tile([128, NT_X], _DT)
            nc.sync.dma_start(do_s[:], dout_pm.ap())
            dish_s = cpool.tile([128, NT_S], _DT)
            nc.sync.dma_start(dish_s[:], dinsh.ap())
            dosh_s = cpool.tile([128, NT_S], _DT)
            nc.sync.dma_start(dosh_s[:], doutsh.ap())

            # ---- zero agg tables + table zero-rows ----
            z = zpool.tile([128, 13, 128], _DT)
            nc.vector.memset(z[:], 0.0)
            a1v = agg1.ap()[:12800, :].rearrange("(a p) e -> p a e", p=128)
            a2v = agg2.ap()[:12800, :].rearrange("(a p) e -> p a e", p=128)
            for a in range(10):
                nc.sync.dma_start(a1v[:, a * 10:(a + 1) * 10, :],
                                  z[:, :10, :])
                nc.sync.dma_start(a2v[:, a * 10:(a + 1) * 10, :],
                                  z[:, :10, :NCLS_PAD])
            nc.sync.dma_start(
                agg1.ap()[12800:, :].rearrange("(a p) e -> p a e", p=128),
                z[:, :6, :])
            nc.sync.dma_start(
                agg2.ap()[12800:, :].rearrange("(a p) e -> p a e", p=128),
                z[:, :6, :NCLS_PAD])
            # zero rows of the tables (row 0; row 50001 of table1 is written
            # as zero by the h-build since x_pad rows >= 50000 are zero and
            # dout_pm pads are zero)
            nc.sync.dma_start(t1ap[0:1, :], z[:1, 0, :])
            nc.sync.dma_start(t2ap[0:1, :], z[:1, 0, :NCLS_PAD])
            nc.sync.dma_start(t2ap[N_NODES + 1:N_NODES + 2, :],
                              z[:1, 0, :NCLS_PAD])

            # ---- h-table build: table1[1+n] = (x @ W1) * d_out ----
            XB = 4  # node tiles per DMA batch
            for tb in range((NT_X + XB - 1) // XB):
                t0 = tb * XB
                nt = min(XB, NT_X - t0)
                xt = xpool.tile([128, XB, 128], _DT, tag="xt")
                nc.sync.dma_start(
                    xt[:, :nt, :].rearrange("p a e -> p (a e)"),
                    xT.ap()[:, t0 * 128:(t0 + nt) * 128],
                )
                hb = hpool.tile([128, XB, 128], _DT, tag="hb")
                for i in range(nt):
                    t = t0 + i
                    hp = mmps.tile([128, HID], _DT)
                    nc.tensor.matmul(hp[:], xt[:, i, :], w1_s[:],
                                     start=True, stop=True)
                    nc.scalar.activation(hb[:, i, :], hp[:],
                                         mybir.ActivationFunctionType.Copy,
                                         scale=do_s[:, t:t + 1])
                nc.sync.dma_start(
                    t1ap[1 + t0 * 128:1 + (t0 + nt) * 128, :].rearrange(
                        "(a p) e -> p a e", p=128),
                    hb[:, :nt, :],
                )

            # ---- aggregation chunks ----
            def agg_chunks(table_ap, agg_ap, elem, bpool, tag):
                goff = 0  # running gather-token offset (in idx cols of 16)
                soff = 0
                for half in range(2):
                    base = half * (HALF_N + 1)
                    src_rows = table_ap[base: base + HALF_N + 1, :]
                    for K in struct[half]:
                        S = sum(K)  # unit slots
                        T = 4 * S  # gather tokens
                        gi = ipool.tile([128, T // 16], mybir.dt.int16,
                                        tag="gi")
                        nc.sync.dma_start(
                            gi[:], gidx.ap()[:, goff:goff + T // 16])
                        goff += T // 16
                        si = ipool.tile([128, G_CAP // 16], mybir.dt.int16,
                                        tag="si")
                        nc.sync.dma_start(
                            si[:], sidx.ap()[:, soff:soff + G_CAP // 16])
                        soff += G_CAP // 16
                        buf = bpool.tile([128, T // 128, elem], _DT, tag=tag)
                        nc.gpsimd.dma_gather(buf[:, :, :], src_rows, gi[:],
                                             T, T, elem, single_packet=False)
                        sc = S // 128
                        # L0: t0 += t1, t2 += t3   ([T0|T2] += [T1|T3])
                        nc.vector.tensor_add(buf[:, :2 * sc, :],
                                             buf[:, :2 * sc, :],
                                             buf[:, 2 * sc:4 * sc, :])
                        # L1: (t0+t1) += (t2+t3)   (T0 += T2)
                        nc.vector.tensor_add(buf[:, :sc, :],
                                             buf[:, :sc, :],
                                             buf[:, sc:2 * sc, :])
                        # chains: U_1 += U_u
                        b_u = K[0] // 128
                        for u in range(1, len(K)):
                            kc = K[u] // 128
                            nc.vector.tensor_add(buf[:, :kc, :],
                                                 buf[:, :kc, :],
                                                 buf[:, b_u:b_u + kc, :])
                            b_u += kc
                        nc.gpsimd.dma_scatter_add(
                            agg_ap[:, :], buf[:, :G_CAP // 128, :], si[:],
                            G_CAP, G_CAP, elem, single_packet=False)

            agg_chunks(t1ap, agg1.ap(), HID, bpool1, "c1")

            # ---- layer-1 post-processing + layer-2 transform ----
            for t in range(NT_S):
                rows = min(128, SHARD - t * 128)
                a0 = ppool.tile([128, HID], _DT, tag="a0")
                nc.sync.dma_start(a0[:], agg1.ap()[t * 128:(t + 1) * 128, :])
                a1 = ppool.tile([128, HID], _DT, tag="a1")
                nc.sync.dma_start(
                    a1[:], agg1.ap()[AGG_H + t * 128:AGG_H + (t + 1) * 128, :])
                nc.vector.tensor_add(a0[:], a0[:], a1[:])
                pp = ppool.tile([128, HID], _DT, tag="pp")
                nc.vector.scalar_tensor_tensor(
                    pp[:], a0[:], dish_s[:, t:t + 1], b1_s[:],
                    op0=mybir.AluOpType.mult, op1=mybir.AluOpType.add)
                rl = ppool.tile([128, HID], _DT, tag="rl")
                nc.scalar.activation(rl[:], pp[:],
                                     mybir.ActivationFunctionType.Relu)
                tp = pps.tile([128, 128], _DT, tag="tp")
                nc.tensor.transpose(tp[:], rl[:], id_s[:])
                h1T = ppool.tile([128, 128], _DT, tag="h1T")
                nc.vector.tensor_copy(h1T[:], tp[:])
                p2p = pps.tile([128, NCLS_PAD], _DT, tag="p2p")
                nc.tensor.matmul(p2p[:], h1T[:], w2_s[:], start=True,
                                 stop=True)
                p2s = ppool.tile([128, NCLS_PAD], _DT, tag="p2s")
                nc.scalar.activation(p2s[:], p2p[:],
                                     mybir.ActivationFunctionType.Copy,
                                     scale=dosh_s[:, t:t + 1])
                nc.sync.dma_start(p2b.ap()[t * 128:t * 128 + rows, :],
                                  p2s[:rows, :])

            # ---- AllGather p2 shards into table2 rows [1, 50001) ----
            nc.gpsimd.collective_compute(
                "AllGather", mybir.AluOpType.bypass,
                replica_groups=[list(range(N_CORES))],
                ins=[p2b.ap()], outs=[t2ap[1:N_NODES + 1, :]],
            )

            # ---- layer-2 aggregation ----
            agg_chunks(t2ap, agg2.ap(), NCLS_PAD, bpool2, "c2")

            # ---- layer-2 post-processing -> output shard ----
            for t in range(NT_S):
                rows = min(128, SHARD - t * 128)
                c0 = ppool.tile([128, NCLS_PAD], _DT, tag="c0")
                nc.sync.dma_start(c0[:], agg2.ap()[t * 128:(t + 1) * 128, :])
                c1 = ppool.tile([128, NCLS_PAD], _DT, tag="c1t")
                nc.sync.dma_start(
                    c1[:], agg2.ap()[AGG_H + t * 128:AGG_H + (t + 1) * 128, :])
                nc.vector.tensor_add(c0[:], c0[:], c1[:])
                o = ppool.tile([128, NCLS_PAD], _DT, tag="o")
                nc.vector.scalar_tensor_tensor(
                    o[:], c0[:], dish_s[:, t:t + 1], b2_s[:],
                    op0=mybir.AluOpType.mult, op1=mybir.AluOpType.add)
                nc.sync.dma_start(out.ap()[t * 128:t * 128 + rows, :],
                                  o[:rows, :NCLS])

    nc.compile()
    return nc


def _preprocess(edge_index: np.ndarray):
    """Host-side sharding. Returns degrees, per-core flat idx arrays, and the
    chunk structure struct[h] = [[K_u ...] per chunk]."""
    src = edge_index[0].astype(np.int64)
    dst = edge_index[1].astype(np.int64)

    deg_out = np.bincount(src, minlength=N_NODES).astype(np.float64)
    deg_in = np.bincount(dst, minlength=N_NODES).astype(np.float64)
    d_out = (np.where(deg_out > 0, deg_out, 1.0) ** -0.5).astype(np.float32)
    d_in = (np.where(deg_in > 0, deg_in, 1.0) ** -0.5).astype(np.float32)

    core = dst // SHARD
    half = src // HALF_N
    sect = core * 2 + half  # 0..15

    # group key: (sect, dst); edges sorted by group
    gkey = sect * N_NODES + dst
    order = np.argsort(gkey, kind="stable")
    s_src, s_dst = src[order], dst[order]
    s_sect = sect[order]
    skey = gkey[order]

    E = len(s_src)
    new_grp = np.r_[True, skey[1:] != skey[:-1]]
    grp_of_edge = np.cumsum(new_grp) - 1
    grp_start = np.flatnonzero(new_grp)
    n_grp = len(grp_start)
    grp_size = np.diff(np.r_[grp_start, E])
    pos_in_grp = np.arange(E) - grp_start[grp_of_edge]

    grp_sect = s_sect[grp_start]
    grp_dst = s_dst[grp_start]
    grp_m = (grp_size + 3) // 4  # units per group

    # rank groups within each section by m desc (stable)
    rank_order = np.lexsort((grp_dst, -grp_m, grp_sect))
    grp_rank = np.empty(n_grp, np.int64)
    # position within section
    rs = grp_sect[rank_order]
    sec_first = np.r_[True, rs[1:] != rs[:-1]]
    within = np.arange(n_grp) - np.maximum.accumulate(
        np.where(sec_first, np.arange(n_grp), 0))
    grp_rank[rank_order] = within

    grp_chunk = grp_rank // G_CAP
    grp_rr = grp_rank % G_CAP

    # per (sect, chunk, u): k_u = #groups with m >= u in that chunk
    n_chunk_sec = np.zeros(16, np.int64)
    np.maximum.at(n_chunk_sec, grp_sect, grp_chunk + 1)
    H = [int(max(n_chunk_sec[h::2].max(), 1)) for h in range(2)]
    # m sorted desc within chunk -> k_u(sect, chunk, u) = count of m >= u
    MMAX = int(grp_m.max())
    ku = np.zeros((16, max(H) if (max(H)) else 1, MMAX), np.int64)
    for u in range(1, MMAX + 1):
        sel = grp_m >= u
        np.add.at(ku, (grp_sect[sel], grp_chunk[sel], u - 1), 1)

    # unify across cores: K_u[h][k] = roundup128(max over cores)
    struct = []
    for h in range(2):
        chunks = []
        for k in range(H[h]):
            kmax = ku[h::2, k, :].max(axis=0)  # [MMAX]
            K = [int(-(-v // 128) * 128) for v in kmax if v > 0]
            if not K:
                K = [128]
            K[0] = G_CAP  # scatter slab always full G_CAP
            # K_u must be non-increasing (prefix property), G_CAP >= all
            for u in range(1, len(K)):
                K[u] = min(K[u], K[u - 1])
            chunks.append(K)
        struct.append(chunks)

    # token offsets per (h, k)
    gbase = {}
    sbase = {}
    go = 0
    so = 0
    for h in range(2):
        for k in range(H[h]):
            gbase[(h, k)] = go
            sbase[(h, k)] = so
            go += 4 * sum(struct[h][k])
            so += G_CAP
    tot_g, tot_s = go, so

    # per-core flat token arrays
    ZVAL = np.array([ZERO_IDX_0, ZERO_IDX_1], np.int16)
    gflat = np.empty((N_CORES, tot_g), np.int16)
    sflat = np.empty((N_CORES, tot_s), np.int16)
    # defaults: gather -> zero row (per half), scatter -> unique trash
    for h in range(2):
        for k in range(H[h]):
            a = gbase[(h, k)]
            gflat[:, a:a + 4 * sum(struct[h][k])] = ZVAL[h]
            b = sbase[(h, k)]
            sflat[:, b:b + G_CAP] = (TRASH0 +
                                     np.arange(G_CAP)).astype(np.int16)

    # slab bases b_u within a chunk (unit slots)
    bu = {}
    for h in range(2):
        for k in range(H[h]):
            K = struct[h][k]
            acc = 0
            for u, Ku in enumerate(K):
                bu[(h, k, u)] = acc
                acc += Ku

    # edge -> token slot
    e_grp = grp_of_edge
    e_sect = s_sect
    e_h = e_sect % 2
    e_core = e_sect // 2
    e_chunk = grp_chunk[e_grp]
    e_rr = grp_rr[e_grp]
    e_u = pos_in_grp // 4  # unit index (0-based)
    e_lane = pos_in_grp % 4
    # S per (h, chunk)
    S_hk = {(h, k): sum(struct[h][k]) for h in range(2)
            for k in range(len(struct[h]))}
    # vectorized: build lookup arrays indexed by (h, chunk)
    max_chunks = max(H)
    S_arr = np.zeros((2, max_chunks), np.int64)
    gb_arr = np.zeros((2, max_chunks), np.int64)
    for h in range(2):
        for k in range(H[h]):
            S_arr[h, k] = S_hk[(h, k)]
            gb_arr[h, k] = gbase[(h, k)]
    bu_arr = np.zeros((2, max_chunks, MMAX), np.int64)
    for (h, k, u), v in bu.items():
        bu_arr[h, k, u] = v

    lane_mult = np.array([0, 2, 1, 3], np.int64)  # t0,t1,t2,t3 region order
    slot_in_chunk = (lane_mult[e_lane] * S_arr[e_h, e_chunk]
                     + bu_arr[e_h, e_chunk, e_u] + e_rr)
    gtok = gb_arr[e_h, e_chunk] + slot_in_chunk
    # idx value: row (1 + src) relative to half base (half*(HALF_N+1))
    e_idx = (1 + s_src - e_h * (HALF_N + 1)).astype(np.int16)
    gflat[e_core, gtok] = e_idx

    # scatter tokens: one per group
    sb_arr = np.zeros((2, max_chunks), np.int64)
    for h in range(2):
        for k in range(H[h]):
            sb_arr[h, k] = sbase[(h, k)]
    g_h = grp_sect % 2
    g_core = grp_sect // 2
    stok = sb_arr[g_h, grp_chunk] + grp_rr
    sval = (g_h * AGG_H + (grp_dst - g_core * SHARD)).astype(np.int16)
    sflat[g_core, stok] = sval

    def wrap(a):
        # token i -> partition i%16, col i//16; replicate x8 to 128 partitions
        n = a.shape[1]
        w = a.reshape(N_CORES, n // 16, 16).transpose(0, 2, 1)
        return np.ascontiguousarray(np.tile(w, (1, 8, 1)))

    return d_out, d_in, wrap(gflat), wrap(sflat), struct


_cache: dict = {}


def _run(inputs: dict, trace: bool = False, trace_cores=None):
    node_embeddings = np.asarray(inputs["node_embeddings"], np.float32)
    W1 = np.asarray(inputs["W1"], np.float32)
    b1 = np.asarray(inputs["b1"], np.float32)
    W2 = np.asarray(inputs["W2"], np.float32)
    b2 = np.asarray(inputs["b2"], np.float32)
    edge_index = np.asarray(inputs["edge_index"])

    d_out, d_in, gflat_w, sflat_w, struct = _preprocess(edge_index)

    key = repr(struct)
    if key not in _cache:
        _cache[key] = _build_program(struct)
    nc = _cache[key]

    x_pad = np.zeros((NPAD, F_IN), np.float32)
    x_pad[:N_NODES] = node_embeddings
    xT = np.ascontiguousarray(x_pad.T)
    W2p = np.zeros((HID, NCLS_PAD), np.float32)
    W2p[:, :NCLS] = W2
    b1bc = np.tile(b1[None, :], (128, 1)).astype(np.float32)
    b2p = np.zeros(NCLS_PAD, np.float32)
    b2p[:NCLS] = b2
    b2bc = np.tile(b2p[None, :], (128, 1)).astype(np.float32)
    ident = np.eye(128, dtype=np.float32)
    do_pad = np.zeros(NPAD, np.float32)
    do_pad[:N_NODES] = d_out
    dout_pm = np.ascontiguousarray(do_pad.reshape(NT_X, 128).T)

    in_maps = []
    for c in range(N_CORES):
        sh = slice(c * SHARD, (c + 1) * SHARD)
        dish_pad = np.zeros(NT_S * 128, np.float32)
        dish_pad[:SHARD] = d_in[sh]
        dosh_pad = np.zeros(NT_S * 128, np.float32)
        dosh_pad[:SHARD] = d_out[sh]
        in_maps.append({
            "xT": xT,
            "W1": W1,
            "W2p": W2p,
            "b1bc": b1bc,
            "b2bc": b2bc,
            "ident": ident,
            "dout_pm": dout_pm,
            "dinsh": np.ascontiguousarray(dish_pad.reshape(NT_S, 128).T),
            "doutsh": np.ascontiguousarray(dosh_pad.reshape(NT_S, 128).T),
            "gidx": gflat_w[c],
            "sidx": sflat_w[c],
        })

    kw = {}
    if trace:
        kw = dict(trace=True,
                  trace_cores=trace_cores if trace_cores else [0])
    res = bass_utils.run_bass_kernel_spmd(
        nc, in_maps, core_ids=list(range(N_CORES)), **kw)
    out = np.concatenate([r["out"] for r in res.results], axis=0)
    return out, res


def kernel(**inputs) -> np.ndarray:
    out, _ = _run(inputs, trace=False)
    return out


# revision 11
# speedup vs baseline: 1.1976x; 1.1976x over previous
"""GCN (2-layer GraphConv) Trainium2 Bass kernel, 8-core SPMD.

Strategy (dst-sharded graph parallel):
- Nodes partitioned into 8 shards of 6250 (core c owns dst nodes [6250c, 6250(c+1))).
- Edges assigned to the core owning their dst; split by src half (int16 gather idx).
- Transform tables replicated: every core computes h = (x @ W1) * d_out for ALL
  nodes into a local DRAM table (rows shifted +1; rows 0 and 50001 are zero, used
  as the gather target for padding tokens).
- Aggregation per core: per (dst, half) group, edges are padded to pairs of 2
  (pads gather the zero row). Groups sorted by unit count m = ceil(deg/2)
  descending; top 768 groups form chunk 0, the rest are dealt round-robin to
  chunks 1..H-1 (token-balanced), sorted by m desc within each chunk. Gathered
  chunk layout [T0 | T1] where T0 = [U_1 | ... | U_M] slabs (U_u = u-th pair of
  each group with m >= u, 128-rounded caps K_u uniform across cores per chunk
  index). One slab add reduces pairs (T0 += T1), then chain slab adds fold U_u
  into U_1. Each group ends as ONE token; dma_scatter_add writes it to row
  half*6400+dst — all rows unique per call and across calls (the instruction
  loses updates on duplicate rows - verified on HW).
- Layer 2: p2 = (relu(sum_halves(agg)*d_in + b1) @ W2pad) * d_out on the owned
  shard, AllGathered into table2 (same +1 row shift), then the same chunk
  structure aggregates 64-float tokens with the same index arrays.
- Output: each core returns its [6250, 32] shard; the host concatenates.
"""

import numpy as np

import concourse.bacc as bacc
import concourse.bass as bass  # noqa: F401
import concourse.mybir as mybir
import concourse.tile as tile
from concourse import bass_utils

N_NODES = 50000
N_CORES = 8
SHARD = 6250
HALF_N = 25000
F_IN = 128
HID = 128
NCLS = 32
NCLS_PAD = 64

NT_X = 391  # ceil(50000/128) node tiles for the h-table build
NPAD = NT_X * 128  # 50048
NT_S = 49  # ceil(6250/128) shard tiles

G_CAP = 768  # groups per chunk (scatter tokens per chunk)
ZERO_IDX_0 = 0  # zero row idx for half 0 (table row 0)
ZERO_IDX_1 = 25000  # zero row idx for half 1 (table row 50001 - base 25001)
AGG_H = 6400  # agg rows per half
TRASH0 = 2 * AGG_H  # trash rows base
AGG_ROWS = TRASH0 + 2048

_DT = mybir.dt.float32


def _build_program(struct):
    """struct[h] = list of chunks; chunk = list of K_u (each a multiple of 128).
    Builds + compiles the 8-core SPMD program."""
    nc = bacc.Bacc("TRN2", target_bir_lowering=False, debug=False,
                   num_devices=N_CORES)

    tot_gtok = sum(2 * sum(ch) for h in range(2) for ch in struct[h])
    tot_stok = sum(ch[0] for h in range(2) for ch in struct[h])

    xT = nc.dram_tensor("xT", [128, NPAD], _DT, kind="ExternalInput")
    W1 = nc.dram_tensor("W1", [F_IN, HID], _DT, kind="ExternalInput")
    W2p = nc.dram_tensor("W2p", [HID, NCLS_PAD], _DT, kind="ExternalInput")
    b1bc = nc.dram_tensor("b1bc", [128, HID], _DT, kind="ExternalInput")
    b2bc = nc.dram_tensor("b2bc", [128, NCLS_PAD], _DT, kind="ExternalInput")
    ident = nc.dram_tensor("ident", [128, 128], _DT, kind="ExternalInput")
    dout_pm = nc.dram_tensor("dout_pm", [128, NT_X], _DT, kind="ExternalInput")
    dinsh = nc.dram_tensor("dinsh", [128, NT_S], _DT, kind="ExternalInput")
    doutsh = nc.dram_tensor("doutsh", [128, NT_S], _DT, kind="ExternalInput")
    gidx = nc.dram_tensor("gidx", [128, tot_gtok // 16], mybir.dt.int16,
                          kind="ExternalInput")
    sidx = nc.dram_tensor("sidx", [128, tot_stok // 16], mybir.dt.int16,
                          kind="ExternalInput")
    out = nc.dram_tensor("out", [SHARD, NCLS], _DT, kind="ExternalOutput")

    # table1 split by half so half-0 gathers only depend on half-0 builds.
    # t1a: row 0 = zero, rows 1+n = node n (n < 25000).
    # t1b: row r = node 25000+r; rows >= 25000 are zero (x pad + dout pad).
    t1a = nc.dram_tensor("t1a", [HALF_N + 1, HID], _DT, kind="Internal")
    t1b = nc.dram_tensor("t1b", [NPAD - HALF_N, HID], _DT, kind="Internal")
    agg1 = nc.dram_tensor("agg1", [AGG_ROWS, HID], _DT, kind="Internal")
    agg2 = nc.dram_tensor("agg2", [AGG_ROWS, NCLS_PAD], _DT, kind="Internal")
    p2b = nc.dram_tensor("p2b", [SHARD, NCLS_PAD], _DT, kind="Internal")
    table2 = nc.dram_tensor("table2", [N_NODES + 2, NCLS_PAD], _DT,
                            kind="Internal", addr_space="Shared")

    t2ap = table2.ap()

    with tile.TileContext(nc) as tc:
        with (
            tc.tile_pool(name="const", bufs=1) as cpool,
            tc.tile_pool(name="xload", bufs=3) as xpool,
            tc.tile_pool(name="hstore", bufs=3) as hpool,
            tc.tile_pool(name="mmps", bufs=2, space="PSUM") as mmps,
            tc.tile_pool(name="idx", bufs=6) as ipool,
            tc.tile_pool(name="buf1", bufs=2) as bpool1,
            tc.tile_pool(name="buf2", bufs=2) as bpool2,
            tc.tile_pool(name="post", bufs=3) as ppool,
            tc.tile_pool(name="pps", bufs=2, space="PSUM") as pps,
            tc.tile_pool(name="zero", bufs=1) as zpool,
        ):
            # ---- constants ----
            w1_s = cpool.tile([F_IN, HID], _DT)
            nc.sync.dma_start(w1_s[:], W1.ap())
            w2_s = cpool.tile([HID, NCLS_PAD], _DT)
            nc.sync.dma_start(w2_s[:], W2p.ap())
            b1_s = cpool.tile([128, HID], _DT)
            nc.sync.dma_start(b1_s[:], b1bc.ap())
            b2_s = cpool.tile([128, NCLS_PAD], _DT)
            nc.sync.dma_start(b2_s[:], b2bc.ap())
            id_s = cpool.tile([128, 128], _DT)
            nc.sync.dma_start(id_s[:], ident.ap())
            do_s = cpool.tile([128, NT_X], _DT)
            nc.sync.dma_start(do_s[:], dout_pm.ap())
            dish_s = cpool.tile([128, NT_S], _DT)
            nc.sync.dma_start(dish_s[:], dinsh.ap())
            dosh_s = cpool.tile([128, NT_S], _DT)
            nc.sync.dma_start(dosh_s[:], doutsh.ap())

            # ---- zero agg tables + table zero-rows ----
            z = zpool.tile([128, 16, 128], _DT)
            nc.vector.memset(z[:], 0.0)
            a1v = agg1.ap()[:12800, :].rearrange("(a p) e -> p a e", p=128)
            a2v = agg2.ap()[:12800, :].rearrange("(a p) e -> p a e", p=128)
            for a in range(10):
                nc.sync.dma_start(a1v[:, a * 10:(a + 1) * 10, :],
                                  z[:, :10, :])
                nc.sync.dma_start(a2v[:, a * 10:(a + 1) * 10, :],
                                  z[:, :10, :NCLS_PAD])
            nc.sync.dma_start(
                agg1.ap()[12800:, :].rearrange("(a p) e -> p a e", p=128),
                z[:, :16, :])
            nc.sync.dma_start(
                agg2.ap()[12800:, :].rearrange("(a p) e -> p a e", p=128),
                z[:, :16, :NCLS_PAD])
            nc.sync.dma_start(t1a.ap()[0:1, :], z[:1, 0, :])
            nc.sync.dma_start(t2ap[0:1, :], z[:1, 0, :NCLS_PAD])
            nc.sync.dma_start(t2ap[N_NODES + 1:N_NODES + 2, :],
                              z[:1, 0, :NCLS_PAD])

            # ---- h-table build: (x @ W1) * d_out -> t1a/t1b ----
            XB = 4  # node tiles per DMA batch

            def table_write(hb, nt, n0):
                """Write hb[:, :nt, :] (nodes n0 .. n0+128*nt) to t1a/t1b."""
                n1 = n0 + 128 * nt
                if n1 <= HALF_N:
                    nc.sync.dma_start(
                        t1a.ap()[1 + n0:1 + n1, :].rearrange(
                            "(a p) e -> p a e", p=128), hb[:, :nt, :])
                elif n0 >= HALF_N:
                    nc.sync.dma_start(
                        t1b.ap()[n0 - HALF_N:n1 - HALF_N, :].rearrange(
                            "(a p) e -> p a e", p=128), hb[:, :nt, :])
                else:
                    # boundary batch: split at node 25000
                    i = (HALF_N - n0) // 128  # full tiles before the split
                    p = (HALF_N - n0) % 128  # partitions of tile i below it
                    if i > 0:
                        nc.sync.dma_start(
                            t1a.ap()[1 + n0:1 + n0 + 128 * i, :].rearrange(
                                "(a p) e -> p a e", p=128), hb[:, :i, :])
                    nc.sync.dma_start(
                        t1a.ap()[1 + n0 + 128 * i:1 + HALF_N, :],
                        hb[:p, i, :])
                    nc.sync.dma_start(t1b.ap()[0:128 - p, :], hb[p:, i, :])
                    if i + 1 < nt:
                        nc.sync.dma_start(
                            t1b.ap()[128 - p:128 - p + 128 * (nt - i - 1), :]
                            .rearrange("(a p) e -> p a e", p=128),
                            hb[:, i + 1:nt, :])

            for tb in range((NT_X + XB - 1) // XB):
                t0 = tb * XB
                nt = min(XB, NT_X - t0)
                xt = xpool.tile([128, XB, 128], _DT, tag="xt")
                nc.sync.dma_start(
                    xt[:, :nt, :].rearrange("p a e -> p (a e)"),
                    xT.ap()[:, t0 * 128:(t0 + nt) * 128],
                )
                hb = hpool.tile([128, XB, 128], _DT, tag="hb")
                for i in range(nt):
                    t = t0 + i
                    hp = mmps.tile([128, HID], _DT)
                    nc.tensor.matmul(hp[:], xt[:, i, :], w1_s[:],
                                     start=True, stop=True)
                    nc.scalar.activation(hb[:, i, :], hp[:],
                                         mybir.ActivationFunctionType.Copy,
                                         scale=do_s[:, t:t + 1])
                table_write(hb, nt, t0 * 128)

            # ---- aggregation chunks (emission software-pipelined) ----
            def agg_chunks(srcs_by_half, agg_ap, elem, bpool, tag):
                goff = 0
                soff = 0
                pending = None  # deferred scatter of the previous chunk
                for half in range(2):
                    src_rows = srcs_by_half[half]
                    for K in struct[half]:
                        S = sum(K)
                        T = 2 * S
                        gi = ipool.tile([128, T // 16], mybir.dt.int16,
                                        tag="gi")
                        nc.sync.dma_start(
                            gi[:], gidx.ap()[:, goff:goff + T // 16])
                        goff += T // 16
                        GK = K[0]
                        si = ipool.tile([128, GK // 16], mybir.dt.int16,
                                        tag="si")
                        nc.sync.dma_start(
                            si[:], sidx.ap()[:, soff:soff + GK // 16])
                        soff += GK // 16
                        buf = bpool.tile([128, T // 128, elem], _DT, tag=tag)
                        nc.gpsimd.dma_gather(buf[:, :, :], src_rows, gi[:],
                                             T, T, elem, single_packet=False)
                        if pending is not None:
                            nc.gpsimd.dma_scatter_add(*pending,
                                                      single_packet=False)
                            pending = None
                        sc = S // 128
                        # L0: T0 += T1
                        nc.vector.tensor_add(buf[:, :sc, :], buf[:, :sc, :],
                                             buf[:, sc:2 * sc, :])
                        # chains: U_1 += U_u
                        b_u = K[0] // 128
                        for u in range(1, len(K)):
                            kc = K[u] // 128
                            nc.vector.tensor_add(buf[:, :kc, :],
                                                 buf[:, :kc, :],
                                                 buf[:, b_u:b_u + kc, :])
                            b_u += kc
                        pending = (agg_ap[:, :], buf[:, :GK // 128, :],
                                   si[:], GK, GK, elem)
                if pending is not None:
                    nc.gpsimd.dma_scatter_add(*pending, single_packet=False)

            agg_chunks([t1a.ap()[0:HALF_N + 1, :], t1b.ap()[0:HALF_N + 1, :]],
                       agg1.ap(), HID, bpool1, "c1")

            # ---- layer-1 post-processing + layer-2 transform ----
            for t in range(NT_S):
                rows = min(128, SHARD - t * 128)
                a0 = ppool.tile([128, HID], _DT, tag="a0")
                nc.sync.dma_start(a0[:], agg1.ap()[t * 128:(t + 1) * 128, :])
                a1 = ppool.tile([128, HID], _DT, tag="a1")
                nc.sync.dma_start(
                    a1[:], agg1.ap()[AGG_H + t * 128:AGG_H + (t + 1) * 128, :])
                nc.vector.tensor_add(a0[:], a0[:], a1[:])
                pp = ppool.tile([128, HID], _DT, tag="pp")
                nc.vector.scalar_tensor_tensor(
                    pp[:], a0[:], dish_s[:, t:t + 1], b1_s[:],
                    op0=mybir.AluOpType.mult, op1=mybir.AluOpType.add)
                rl = ppool.tile([128, HID], _DT, tag="rl")
                nc.scalar.activation(rl[:], pp[:],
                                     mybir.ActivationFunctionType.Relu)
                tp = pps.tile([128, 128], _DT, tag="tp")
                nc.tensor.transpose(tp[:], rl[:], id_s[:])
                h1T = ppool.tile([128, 128], _DT, tag="h1T")
                nc.vector.tensor_copy(h1T[:], tp[:])
                p2p = pps.tile([128, NCLS_PAD], _DT, tag="p2p")
                nc.tensor.matmul(p2p[:], h1T[:], w2_s[:], start=True,
                                 stop=True)
                p2s = ppool.tile([128, NCLS_PAD], _DT, tag="p2s")
                nc.scalar.activation(p2s[:], p2p[:],
                                     mybir.ActivationFunctionType.Copy,
                                     scale=dosh_s[:, t:t + 1])
                nc.sync.dma_start(p2b.ap()[t * 128:t * 128 + rows, :],
                                  p2s[:rows, :])

            # ---- AllGather p2 shards into table2 rows [1, 50001) ----
            nc.gpsimd.collective_compute(
                "AllGather", mybir.AluOpType.bypass,
                replica_groups=[list(range(N_CORES))],
                ins=[p2b.ap()], outs=[t2ap[1:N_NODES + 1, :]],
            )

            # ---- layer-2 aggregation ----
            agg_chunks([t2ap[0:HALF_N + 1, :],
                        t2ap[HALF_N + 1:2 * HALF_N + 2, :]],
                       agg2.ap(), NCLS_PAD, bpool2, "c2")

            # ---- layer-2 post-processing -> output shard ----
            for t in range(NT_S):
                rows = min(128, SHARD - t * 128)
                c0 = ppool.tile([128, NCLS_PAD], _DT, tag="c0")
                nc.sync.dma_start(c0[:], agg2.ap()[t * 128:(t + 1) * 128, :])
                c1 = ppool.tile([128, NCLS_PAD], _DT, tag="c1t")
                nc.sync.dma_start(
                    c1[:], agg2.ap()[AGG_H + t * 128:AGG_H + (t + 1) * 128, :])
                nc.vector.tensor_add(c0[:], c0[:], c1[:])
                o = ppool.tile([128, NCLS_PAD], _DT, tag="o")
                nc.vector.scalar_tensor_tensor(
                    o[:], c0[:], dish_s[:, t:t + 1], b2_s[:],
                    op0=mybir.AluOpType.mult, op1=mybir.AluOpType.add)
                nc.sync.dma_start(out.ap()[t * 128:t * 128 + rows, :],
                                  o[:rows, :NCLS])

    nc.compile()
    return nc


def _preprocess(edge_index: np.ndarray):
    """Host-side sharding. Returns degrees, per-core flat idx arrays, and the
    chunk structure struct[h] = [[K_u ...] per chunk]. Units are PAIRS of
    same-(dst,half) edges (odd groups padded with a zero-row gather)."""
    src = edge_index[0].astype(np.int64)
    dst = edge_index[1].astype(np.int64)

    deg_out = np.bincount(src, minlength=N_NODES).astype(np.float64)
    deg_in = np.bincount(dst, minlength=N_NODES).astype(np.float64)
    d_out = (np.where(deg_out > 0, deg_out, 1.0) ** -0.5).astype(np.float32)
    d_in = (np.where(deg_in > 0, deg_in, 1.0) ** -0.5).astype(np.float32)

    core = dst // SHARD
    half = src // HALF_N
    sect = core * 2 + half  # 0..15

    gkey = sect * N_NODES + dst
    order = np.argsort(gkey, kind="stable")
    s_src, s_dst = src[order], dst[order]
    s_sect = sect[order]
    skey = gkey[order]

    E = len(s_src)
    new_grp = np.r_[True, skey[1:] != skey[:-1]]
    grp_of_edge = np.cumsum(new_grp) - 1
    grp_start = np.flatnonzero(new_grp)
    n_grp = len(grp_start)
    grp_size = np.diff(np.r_[grp_start, E])
    pos_in_grp = np.arange(E) - grp_start[grp_of_edge]

    grp_sect = s_sect[grp_start]
    grp_dst = s_dst[grp_start]
    grp_m = (grp_size + 1) // 2  # pair units per group

    # order groups within each section by m desc (stable by dst)
    rank_order = np.lexsort((grp_dst, -grp_m, grp_sect))
    grp_rank = np.empty(n_grp, np.int64)
    rs = grp_sect[rank_order]
    sec_first = np.r_[True, rs[1:] != rs[:-1]]
    within = np.arange(n_grp) - np.maximum.accumulate(
        np.where(sec_first, np.arange(n_grp), 0))
    grp_rank[rank_order] = within

    # chunk assignment: consecutive groups in m-desc rank order, packed by a
    # token budget (keeps chunks m-homogeneous -> minimal slab rounding, and
    # token-balanced -> bounded SBUF buffers). Within each chunk rank order
    # stays m-desc (prefix property for the chain slabs).
    T_BUDGET = 6144
    # exclusive prefix of 2*m within each section (rank order)
    ro = rank_order  # groups ordered (sect, m desc)
    m_ro = grp_m[ro]
    cum = np.cumsum(2 * m_ro) - 2 * m_ro
    rs2 = grp_sect[ro]
    first = np.r_[True, rs2[1:] != rs2[:-1]]
    sec_base = np.maximum.accumulate(np.where(first, cum, 0))
    chunk_ro = (cum - sec_base) // T_BUDGET
    grp_chunk = np.empty(n_grp, np.int64)
    grp_chunk[ro] = chunk_ro
    # rank within chunk
    sc_key = rs2 * 64 + chunk_ro
    cfirst = np.r_[True, sc_key[1:] != sc_key[:-1]]
    widx = np.arange(n_grp)
    rr_ro = widx - np.maximum.accumulate(np.where(cfirst, widx, 0))
    grp_rr = np.empty(n_grp, np.int64)
    grp_rr[ro] = rr_ro
    H = int(grp_chunk.max()) + 1

    # k_u(sect, chunk, u) = #groups with m >= u
    MMAX = int(grp_m.max())
    Hmax = H
    ku = np.zeros((16, Hmax, MMAX), np.int64)
    for u in range(1, MMAX + 1):
        sel = grp_m >= u
        np.add.at(ku, (grp_sect[sel], grp_chunk[sel], u - 1), 1)

    struct = []
    for h in range(2):
        chunks = []
        for k in range(H):
            kmax = ku[h::2, k, :].max(axis=0)
            K = [int(-(-v // 128) * 128) for v in kmax if v > 0]
            if not K:
                K = [128]
            for u in range(1, len(K)):
                K[u] = min(K[u], K[u - 1])
            chunks.append(K)
        struct.append(chunks)

    # offsets
    gbase = np.zeros((2, H), np.int64)
    sbase = np.zeros((2, H), np.int64)
    go = 0
    so = 0
    for h in range(2):
        for k in range(H):
            gbase[h, k] = go
            sbase[h, k] = so
            go += 2 * sum(struct[h][k])
            so += struct[h][k][0]
    tot_g, tot_s = go, so

    ZVAL = np.array([ZERO_IDX_0, ZERO_IDX_1], np.int16)
    gflat = np.empty((N_CORES, tot_g), np.int16)
    sflat = np.empty((N_CORES, tot_s), np.int16)
    for h in range(2):
        for k in range(H):
            a = gbase[h, k]
            gflat[:, a:a + 2 * sum(struct[h][k])] = ZVAL[h]
            b = sbase[h, k]
            gk = struct[h][k][0]
            sflat[:, b:b + gk] = (TRASH0 + np.arange(gk)).astype(np.int16)

    # slab bases
    bu_arr = np.zeros((2, H, MMAX), np.int64)
    S_arr = np.zeros((2, H), np.int64)
    for h in range(2):
        for k in range(H):
            K = struct[h][k]
            acc = 0
            for u, Ku in enumerate(K):
                bu_arr[h, k, u] = acc
                acc += Ku
            S_arr[h, k] = acc

    e_h = s_sect % 2
    e_core = s_sect // 2
    e_chunk = grp_chunk[grp_of_edge]
    e_rr = grp_rr[grp_of_edge]
    e_u = pos_in_grp // 2
    e_lane = pos_in_grp % 2
    slot_in_chunk = (e_lane * S_arr[e_h, e_chunk]
                     + bu_arr[e_h, e_chunk, e_u] + e_rr)
    gtok = gbase[e_h, e_chunk] + slot_in_chunk
    # idx value: half 0 -> row 1+src in t1a view; half 1 -> row src-25000
    # in t1b view (same values work for table2's shifted views)
    e_idx = np.where(e_h == 0, 1 + s_src, s_src - HALF_N).astype(np.int16)
    gflat[e_core, gtok] = e_idx

    g_h = grp_sect % 2
    g_core = grp_sect // 2
    stok = sbase[g_h, grp_chunk] + grp_rr
    sval = (g_h * AGG_H + (grp_dst - g_core * SHARD)).astype(np.int16)
    sflat[g_core, stok] = sval

    def wrap(a):
        n = a.shape[1]
        w = a.reshape(N_CORES, n // 16, 16).transpose(0, 2, 1)
        return np.ascontiguousarray(np.tile(w, (1, 8, 1)))

    return d_out, d_in, wrap(gflat), wrap(sflat), struct


_cache: dict = {}


def _run(inputs: dict, trace: bool = False, trace_cores=None):
    node_embeddings = np.asarray(inputs["node_embeddings"], np.float32)
    W1 = np.asarray(inputs["W1"], np.float32)
    b1 = np.asarray(inputs["b1"], np.float32)
    W2 = np.asarray(inputs["W2"], np.float32)
    b2 = np.asarray(inputs["b2"], np.float32)
    edge_index = np.asarray(inputs["edge_index"])

    d_out, d_in, gflat_w, sflat_w, struct = _preprocess(edge_index)

    key = repr(struct)
    if key not in _cache:
        _cache[key] = _build_program(struct)
    nc = _cache[key]

    x_pad = np.zeros((NPAD, F_IN), np.float32)
    x_pad[:N_NODES] = node_embeddings
    xT = np.ascontiguousarray(x_pad.T)
    W2p = np.zeros((HID, NCLS_PAD), np.float32)
    W2p[:, :NCLS] = W2
    b1bc = np.tile(b1[None, :], (128, 1)).astype(np.float32)
    b2p = np.zeros(NCLS_PAD, np.float32)
    b2p[:NCLS] = b2
    b2bc = np.tile(b2p[None, :], (128, 1)).astype(np.float32)
    ident = np.eye(128, dtype=np.float32)
    do_pad = np.zeros(NPAD, np.float32)
    do_pad[:N_NODES] = d_out
    dout_pm = np.ascontiguousarray(do_pad.reshape(NT_X, 128).T)

    in_maps = []
    for c in range(N_CORES):
        sh = slice(c * SHARD, (c + 1) * SHARD)
        dish_pad = np.zeros(NT_S * 128, np.float32)
        dish_pad[:SHARD] = d_in[sh]
        dosh_pad = np.zeros(NT_S * 128, np.float32)
        dosh_pad[:SHARD] = d_out[sh]
        in_maps.append({
            "xT": xT,
            "W1": W1,
            "W2p": W2p,
            "b1bc": b1bc,
            "b2bc": b2bc,
            "ident": ident,
            "dout_pm": dout_pm,
            "dinsh": np.ascontiguousarray(dish_pad.reshape(NT_S, 128).T),
            "doutsh": np.ascontiguousarray(dosh_pad.reshape(NT_S, 128).T),
            "gidx": gflat_w[c],
            "sidx": sflat_w[c],
        })

    kw = {}
    if trace:
        kw = dict(trace=True,
                  trace_cores=trace_cores if trace_cores else [0])
    res = bass_utils.run_bass_kernel_spmd(
        nc, in_maps, core_ids=list(range(N_CORES)), **kw)
    out = np.concatenate([r["out"] for r in res.results], axis=0)
    return out, res


def kernel(**inputs) -> np.ndarray:
    out, _ = _run(inputs, trace=False)
    return out


# revision 12
# speedup vs baseline: 1.1985x; 1.0008x over previous
"""GCN (2-layer GraphConv) Trainium2 Bass kernel, 8-core SPMD.

Strategy (dst-sharded graph parallel):
- Nodes partitioned into 8 shards of 6250 (core c owns dst nodes [6250c, 6250(c+1))).
- Edges assigned to the core owning their dst; split by src half (int16 gather idx).
- Transform tables replicated: every core computes h = (x @ W1) * d_out for ALL
  nodes into a local DRAM table (rows shifted +1; rows 0 and 50001 are zero, used
  as the gather target for padding tokens).
- Aggregation per core: per (dst, half) group, edges are padded to pairs of 2
  (pads gather the zero row). Groups sorted by unit count m = ceil(deg/2)
  descending; top 768 groups form chunk 0, the rest are dealt round-robin to
  chunks 1..H-1 (token-balanced), sorted by m desc within each chunk. Gathered
  chunk layout [T0 | T1] where T0 = [U_1 | ... | U_M] slabs (U_u = u-th pair of
  each group with m >= u, 128-rounded caps K_u uniform across cores per chunk
  index). One slab add reduces pairs (T0 += T1), then chain slab adds fold U_u
  into U_1. Each group ends as ONE token; dma_scatter_add writes it to row
  half*6400+dst — all rows unique per call and across calls (the instruction
  loses updates on duplicate rows - verified on HW).
- Layer 2: p2 = (relu(sum_halves(agg)*d_in + b1) @ W2pad) * d_out on the owned
  shard, AllGathered into table2 (same +1 row shift), then the same chunk
  structure aggregates 64-float tokens with the same index arrays.
- Output: each core returns its [6250, 32] shard; the host concatenates.
"""

import numpy as np

import concourse.bacc as bacc
import concourse.bass as bass  # noqa: F401
import concourse.mybir as mybir
import concourse.tile as tile
from concourse import bass_utils

N_NODES = 50000
N_CORES = 8
SHARD = 6250
HALF_N = 25000
F_IN = 128
HID = 128
NCLS = 32
NCLS_PAD = 64

NT_X = 391  # ceil(50000/128) node tiles for the h-table build
NPAD = NT_X * 128  # 50048
NT_S = 49  # ceil(6250/128) shard tiles

G_CAP = 768  # groups per chunk (scatter tokens per chunk)
ZERO_IDX_0 = 0  # zero row idx for half 0 (table row 0)
ZERO_IDX_1 = 25000  # zero row idx for half 1 (table row 50001 - base 25001)
AGG_H = 6400  # agg rows per half
TRASH0 = 2 * AGG_H  # trash rows base
AGG_ROWS = TRASH0 + 2048

_DT = mybir.dt.float32


def _build_program(struct):
    """struct[h] = list of chunks; chunk = list of K_u (each a multiple of 128).
    Builds + compiles the 8-core SPMD program."""
    nc = bacc.Bacc("TRN2", target_bir_lowering=False, debug=False,
                   num_devices=N_CORES)

    tot_gtok = sum(2 * sum(ch) for h in range(2) for ch in struct[h])
    tot_stok = sum(ch[0] for h in range(2) for ch in struct[h])

    xT = nc.dram_tensor("xT", [128, NPAD], _DT, kind="ExternalInput")
    W1 = nc.dram_tensor("W1", [F_IN, HID], _DT, kind="ExternalInput")
    W2p = nc.dram_tensor("W2p", [HID, NCLS_PAD], _DT, kind="ExternalInput")
    b1bc = nc.dram_tensor("b1bc", [128, HID], _DT, kind="ExternalInput")
    b2bc = nc.dram_tensor("b2bc", [128, NCLS_PAD], _DT, kind="ExternalInput")
    ident = nc.dram_tensor("ident", [128, 128], _DT, kind="ExternalInput")
    dout_pm = nc.dram_tensor("dout_pm", [128, NT_X], _DT, kind="ExternalInput")
    dinsh = nc.dram_tensor("dinsh", [128, NT_S], _DT, kind="ExternalInput")
    doutsh = nc.dram_tensor("doutsh", [128, NT_S], _DT, kind="ExternalInput")
    gidx = nc.dram_tensor("gidx", [128, tot_gtok // 16], mybir.dt.int16,
                          kind="ExternalInput")
    sidx = nc.dram_tensor("sidx", [128, tot_stok // 16], mybir.dt.int16,
                          kind="ExternalInput")
    out = nc.dram_tensor("out", [SHARD, NCLS], _DT, kind="ExternalOutput")

    # table1 split by half so half-0 gathers only depend on half-0 builds.
    # t1a: row 0 = zero, rows 1+n = node n (n < 25000).
    # t1b: row r = node 25000+r; rows >= 25000 are zero (x pad + dout pad).
    t1a = nc.dram_tensor("t1a", [HALF_N + 1, HID], _DT, kind="Internal")
    t1b = nc.dram_tensor("t1b", [NPAD - HALF_N, HID], _DT, kind="Internal")
    agg1 = nc.dram_tensor("agg1", [AGG_ROWS, HID], _DT, kind="Internal")
    agg2 = nc.dram_tensor("agg2", [AGG_ROWS, NCLS_PAD], _DT, kind="Internal")
    p2b = nc.dram_tensor("p2b", [SHARD, NCLS_PAD], _DT, kind="Internal")
    table2 = nc.dram_tensor("table2", [N_NODES + 2, NCLS_PAD], _DT,
                            kind="Internal", addr_space="Shared")

    t2ap = table2.ap()

    with tile.TileContext(nc) as tc:
        with (
            tc.tile_pool(name="const", bufs=1) as cpool,
            tc.tile_pool(name="xload", bufs=3) as xpool,
            tc.tile_pool(name="hstore", bufs=3) as hpool,
            tc.tile_pool(name="mmps", bufs=2, space="PSUM") as mmps,
            tc.tile_pool(name="idx", bufs=6) as ipool,
            tc.tile_pool(name="buf1", bufs=3) as bpool1,
            tc.tile_pool(name="buf2", bufs=3) as bpool2,
            tc.tile_pool(name="post", bufs=3) as ppool,
            tc.tile_pool(name="pps", bufs=2, space="PSUM") as pps,
            tc.tile_pool(name="zero", bufs=1) as zpool,
        ):
            # ---- constants ----
            w1_s = cpool.tile([F_IN, HID], _DT)
            nc.sync.dma_start(w1_s[:], W1.ap())
            w2_s = cpool.tile([HID, NCLS_PAD], _DT)
            nc.sync.dma_start(w2_s[:], W2p.ap())
            b1_s = cpool.tile([128, HID], _DT)
            nc.sync.dma_start(b1_s[:], b1bc.ap())
            b2_s = cpool.tile([128, NCLS_PAD], _DT)
            nc.sync.dma_start(b2_s[:], b2bc.ap())
            id_s = cpool.tile([128, 128], _DT)
            nc.sync.dma_start(id_s[:], ident.ap())
            do_s = cpool.tile([128, NT_X], _DT)
            nc.sync.dma_start(do_s[:], dout_pm.ap())
            dish_s = cpool.tile([128, NT_S], _DT)
            nc.sync.dma_start(dish_s[:], dinsh.ap())
            dosh_s = cpool.tile([128, NT_S], _DT)
            nc.sync.dma_start(dosh_s[:], doutsh.ap())

            # ---- zero agg tables + table zero-rows ----
            z = zpool.tile([128, 16, 128], _DT)
            nc.vector.memset(z[:], 0.0)
            a1v = agg1.ap()[:12800, :].rearrange("(a p) e -> p a e", p=128)
            a2v = agg2.ap()[:12800, :].rearrange("(a p) e -> p a e", p=128)
            for a in range(10):
                nc.sync.dma_start(a1v[:, a * 10:(a + 1) * 10, :],
                                  z[:, :10, :])
                nc.sync.dma_start(a2v[:, a * 10:(a + 1) * 10, :],
                                  z[:, :10, :NCLS_PAD])
            nc.sync.dma_start(
                agg1.ap()[12800:, :].rearrange("(a p) e -> p a e", p=128),
                z[:, :16, :])
            nc.sync.dma_start(
                agg2.ap()[12800:, :].rearrange("(a p) e -> p a e", p=128),
                z[:, :16, :NCLS_PAD])
            nc.sync.dma_start(t1a.ap()[0:1, :], z[:1, 0, :])
            nc.sync.dma_start(t2ap[0:1, :], z[:1, 0, :NCLS_PAD])
            nc.sync.dma_start(t2ap[N_NODES + 1:N_NODES + 2, :],
                              z[:1, 0, :NCLS_PAD])

            # ---- h-table build: (x @ W1) * d_out -> t1a/t1b ----
            XB = 4  # node tiles per DMA batch

            def table_write(hb, nt, n0):
                """Write hb[:, :nt, :] (nodes n0 .. n0+128*nt) to t1a/t1b."""
                n1 = n0 + 128 * nt
                if n1 <= HALF_N:
                    nc.sync.dma_start(
                        t1a.ap()[1 + n0:1 + n1, :].rearrange(
                            "(a p) e -> p a e", p=128), hb[:, :nt, :])
                elif n0 >= HALF_N:
                    nc.sync.dma_start(
                        t1b.ap()[n0 - HALF_N:n1 - HALF_N, :].rearrange(
                            "(a p) e -> p a e", p=128), hb[:, :nt, :])
                else:
                    # boundary batch: split at node 25000
                    i = (HALF_N - n0) // 128  # full tiles before the split
                    p = (HALF_N - n0) % 128  # partitions of tile i below it
                    if i > 0:
                        nc.sync.dma_start(
                            t1a.ap()[1 + n0:1 + n0 + 128 * i, :].rearrange(
                                "(a p) e -> p a e", p=128), hb[:, :i, :])
                    nc.sync.dma_start(
                        t1a.ap()[1 + n0 + 128 * i:1 + HALF_N, :],
                        hb[:p, i, :])
                    nc.sync.dma_start(t1b.ap()[0:128 - p, :], hb[p:, i, :])
                    if i + 1 < nt:
                        nc.sync.dma_start(
                            t1b.ap()[128 - p:128 - p + 128 * (nt - i - 1), :]
                            .rearrange("(a p) e -> p a e", p=128),
                            hb[:, i + 1:nt, :])

            for tb in range((NT_X + XB - 1) // XB):
                t0 = tb * XB
                nt = min(XB, NT_X - t0)
                xt = xpool.tile([128, XB, 128], _DT, tag="xt")
                nc.sync.dma_start(
                    xt[:, :nt, :].rearrange("p a e -> p (a e)"),
                    xT.ap()[:, t0 * 128:(t0 + nt) * 128],
                )
                hb = hpool.tile([128, XB, 128], _DT, tag="hb")
                for i in range(nt):
                    t = t0 + i
                    hp = mmps.tile([128, HID], _DT)
                    nc.tensor.matmul(hp[:], xt[:, i, :], w1_s[:],
                                     start=True, stop=True)
                    nc.scalar.activation(hb[:, i, :], hp[:],
                                         mybir.ActivationFunctionType.Copy,
                                         scale=do_s[:, t:t + 1])
                table_write(hb, nt, t0 * 128)

            # ---- aggregation chunks (emission software-pipelined) ----
            def agg_chunks(srcs_by_half, agg_ap, elem, bpool, tag):
                goff = 0
                soff = 0
                pending = None  # deferred scatter of the previous chunk
                for half in range(2):
                    src_rows = srcs_by_half[half]
                    for K in struct[half]:
                        S = sum(K)
                        T = 2 * S
                        gi = ipool.tile([128, T // 16], mybir.dt.int16,
                                        tag="gi")
                        nc.sync.dma_start(
                            gi[:], gidx.ap()[:, goff:goff + T // 16])
                        goff += T // 16
                        GK = K[0]
                        si = ipool.tile([128, GK // 16], mybir.dt.int16,
                                        tag="si")
                        nc.sync.dma_start(
                            si[:], sidx.ap()[:, soff:soff + GK // 16])
                        soff += GK // 16
                        buf = bpool.tile([128, T // 128, elem], _DT, tag=tag)
                        nc.gpsimd.dma_gather(buf[:, :, :], src_rows, gi[:],
                                             T, T, elem, single_packet=False)
                        if pending is not None:
                            nc.gpsimd.dma_scatter_add(*pending,
                                                      single_packet=False)
                            pending = None
                        sc = S // 128
                        # L0: T0 += T1
                        nc.vector.tensor_add(buf[:, :sc, :], buf[:, :sc, :],
                                             buf[:, sc:2 * sc, :])
                        # chains: U_1 += U_u
                        b_u = K[0] // 128
                        for u in range(1, len(K)):
                            kc = K[u] // 128
                            nc.vector.tensor_add(buf[:, :kc, :],
                                                 buf[:, :kc, :],
                                                 buf[:, b_u:b_u + kc, :])
                            b_u += kc
                        pending = (agg_ap[:, :], buf[:, :GK // 128, :],
                                   si[:], GK, GK, elem)
                if pending is not None:
                    nc.gpsimd.dma_scatter_add(*pending, single_packet=False)

            agg_chunks([t1a.ap()[0:HALF_N + 1, :], t1b.ap()[0:HALF_N + 1, :]],
                       agg1.ap(), HID, bpool1, "c1")

            # ---- layer-1 post-processing + layer-2 transform ----
            for t in range(NT_S):
                rows = min(128, SHARD - t * 128)
                a0 = ppool.tile([128, HID], _DT, tag="a0")
                nc.sync.dma_start(a0[:], agg1.ap()[t * 128:(t + 1) * 128, :])
                a1 = ppool.tile([128, HID], _DT, tag="a1")
                nc.sync.dma_start(
                    a1[:], agg1.ap()[AGG_H + t * 128:AGG_H + (t + 1) * 128, :])
                nc.vector.tensor_add(a0[:], a0[:], a1[:])
                pp = ppool.tile([128, HID], _DT, tag="pp")
                nc.vector.scalar_tensor_tensor(
                    pp[:], a0[:], dish_s[:, t:t + 1], b1_s[:],
                    op0=mybir.AluOpType.mult, op1=mybir.AluOpType.add)
                rl = ppool.tile([128, HID], _DT, tag="rl")
                nc.scalar.activation(rl[:], pp[:],
                                     mybir.ActivationFunctionType.Relu)
                tp = pps.tile([128, 128], _DT, tag="tp")
                nc.tensor.transpose(tp[:], rl[:], id_s[:])
                h1T = ppool.tile([128, 128], _DT, tag="h1T")
                nc.vector.tensor_copy(h1T[:], tp[:])
                p2p = pps.tile([128, NCLS_PAD], _DT, tag="p2p")
                nc.tensor.matmul(p2p[:], h1T[:], w2_s[:], start=True,
                                 stop=True)
                p2s = ppool.tile([128, NCLS_PAD], _DT, tag="p2s")
                nc.scalar.activation(p2s[:], p2p[:],
                                     mybir.ActivationFunctionType.Copy,
                                     scale=dosh_s[:, t:t + 1])
                nc.sync.dma_start(p2b.ap()[t * 128:t * 128 + rows, :],
                                  p2s[:rows, :])

            # ---- AllGather p2 shards into table2 rows [1, 50001) ----
            nc.gpsimd.collective_compute(
                "AllGather", mybir.AluOpType.bypass,
                replica_groups=[list(range(N_CORES))],
                ins=[p2b.ap()], outs=[t2ap[1:N_NODES + 1, :]],
            )

            # ---- layer-2 aggregation ----
            agg_chunks([t2ap[0:HALF_N + 1, :],
                        t2ap[HALF_N + 1:2 * HALF_N + 2, :]],
                       agg2.ap(), NCLS_PAD, bpool2, "c2")

            # ---- layer-2 post-processing -> output shard ----
            for t in range(NT_S):
                rows = min(128, SHARD - t * 128)
                c0 = ppool.tile([128, NCLS_PAD], _DT, tag="c0")
                nc.sync.dma_start(c0[:], agg2.ap()[t * 128:(t + 1) * 128, :])
                c1 = ppool.tile([128, NCLS_PAD], _DT, tag="c1t")
                nc.sync.dma_start(
                    c1[:], agg2.ap()[AGG_H + t * 128:AGG_H + (t + 1) * 128, :])
                nc.vector.tensor_add(c0[:], c0[:], c1[:])
                o = ppool.tile([128, NCLS_PAD], _DT, tag="o")
                nc.vector.scalar_tensor_tensor(
                    o[:], c0[:], dish_s[:, t:t + 1], b2_s[:],
                    op0=mybir.AluOpType.mult, op1=mybir.AluOpType.add)
                nc.sync.dma_start(out.ap()[t * 128:t * 128 + rows, :],
                                  o[:rows, :NCLS])

    nc.compile()
    return nc


def _preprocess(edge_index: np.ndarray):
    """Host-side sharding. Returns degrees, per-core flat idx arrays, and the
    chunk structure struct[h] = [[K_u ...] per chunk]. Units are PAIRS of
    same-(dst,half) edges (odd groups padded with a zero-row gather)."""
    src = edge_index[0].astype(np.int64)
    dst = edge_index[1].astype(np.int64)

    deg_out = np.bincount(src, minlength=N_NODES).astype(np.float64)
    deg_in = np.bincount(dst, minlength=N_NODES).astype(np.float64)
    d_out = (np.where(deg_out > 0, deg_out, 1.0) ** -0.5).astype(np.float32)
    d_in = (np.where(deg_in > 0, deg_in, 1.0) ** -0.5).astype(np.float32)

    core = dst // SHARD
    half = src // HALF_N
    sect = core * 2 + half  # 0..15

    gkey = sect * N_NODES + dst
    order = np.argsort(gkey, kind="stable")
    s_src, s_dst = src[order], dst[order]
    s_sect = sect[order]
    skey = gkey[order]

    E = len(s_src)
    new_grp = np.r_[True, skey[1:] != skey[:-1]]
    grp_of_edge = np.cumsum(new_grp) - 1
    grp_start = np.flatnonzero(new_grp)
    n_grp = len(grp_start)
    grp_size = np.diff(np.r_[grp_start, E])
    pos_in_grp = np.arange(E) - grp_start[grp_of_edge]

    grp_sect = s_sect[grp_start]
    grp_dst = s_dst[grp_start]
    grp_m = (grp_size + 1) // 2  # pair units per group

    # order groups within each section by m desc (stable by dst)
    rank_order = np.lexsort((grp_dst, -grp_m, grp_sect))
    grp_rank = np.empty(n_grp, np.int64)
    rs = grp_sect[rank_order]
    sec_first = np.r_[True, rs[1:] != rs[:-1]]
    within = np.arange(n_grp) - np.maximum.accumulate(
        np.where(sec_first, np.arange(n_grp), 0))
    grp_rank[rank_order] = within

    # chunk assignment: consecutive groups in m-desc rank order, packed by a
    # token budget (keeps chunks m-homogeneous -> minimal slab rounding, and
    # token-balanced -> bounded SBUF buffers). Within each chunk rank order
    # stays m-desc (prefix property for the chain slabs).
    T_BUDGET = 4096
    # exclusive prefix of 2*m within each section (rank order)
    ro = rank_order  # groups ordered (sect, m desc)
    m_ro = grp_m[ro]
    cum = np.cumsum(2 * m_ro) - 2 * m_ro
    rs2 = grp_sect[ro]
    first = np.r_[True, rs2[1:] != rs2[:-1]]
    sec_base = np.maximum.accumulate(np.where(first, cum, 0))
    chunk_ro = (cum - sec_base) // T_BUDGET
    grp_chunk = np.empty(n_grp, np.int64)
    grp_chunk[ro] = chunk_ro
    # rank within chunk
    sc_key = rs2 * 64 + chunk_ro
    cfirst = np.r_[True, sc_key[1:] != sc_key[:-1]]
    widx = np.arange(n_grp)
    rr_ro = widx - np.maximum.accumulate(np.where(cfirst, widx, 0))
    grp_rr = np.empty(n_grp, np.int64)
    grp_rr[ro] = rr_ro
    H = int(grp_chunk.max()) + 1

    # k_u(sect, chunk, u) = #groups with m >= u
    MMAX = int(grp_m.max())
    Hmax = H
    ku = np.zeros((16, Hmax, MMAX), np.int64)
    for u in range(1, MMAX + 1):
        sel = grp_m >= u
        np.add.at(ku, (grp_sect[sel], grp_chunk[sel], u - 1), 1)

    struct = []
    for h in range(2):
        chunks = []
        for k in range(H):
            kmax = ku[h::2, k, :].max(axis=0)
            K = [int(-(-v // 128) * 128) for v in kmax if v > 0]
            if not K:
                K = [128]
            for u in range(1, len(K)):
                K[u] = min(K[u], K[u - 1])
            chunks.append(K)
        struct.append(chunks)

    # offsets
    gbase = np.zeros((2, H), np.int64)
    sbase = np.zeros((2, H), np.int64)
    go = 0
    so = 0
    for h in range(2):
        for k in range(H):
            gbase[h, k] = go
            sbase[h, k] = so
            go += 2 * sum(struct[h][k])
            so += struct[h][k][0]
    tot_g, tot_s = go, so

    ZVAL = np.array([ZERO_IDX_0, ZERO_IDX_1], np.int16)
    gflat = np.empty((N_CORES, tot_g), np.int16)
    sflat = np.empty((N_CORES, tot_s), np.int16)
    for h in range(2):
        for k in range(H):
            a = gbase[h, k]
            gflat[:, a:a + 2 * sum(struct[h][k])] = ZVAL[h]
            b = sbase[h, k]
            gk = struct[h][k][0]
            sflat[:, b:b + gk] = (TRASH0 + np.arange(gk)).astype(np.int16)

    # slab bases
    bu_arr = np.zeros((2, H, MMAX), np.int64)
    S_arr = np.zeros((2, H), np.int64)
    for h in range(2):
        for k in range(H):
            K = struct[h][k]
            acc = 0
            for u, Ku in enumerate(K):
                bu_arr[h, k, u] = acc
                acc += Ku
            S_arr[h, k] = acc

    e_h = s_sect % 2
    e_core = s_sect // 2
    e_chunk = grp_chunk[grp_of_edge]
    e_rr = grp_rr[grp_of_edge]
    e_u = pos_in_grp // 2
    e_lane = pos_in_grp % 2
    slot_in_chunk = (e_lane * S_arr[e_h, e_chunk]
                     + bu_arr[e_h, e_chunk, e_u] + e_rr)
    gtok = gbase[e_h, e_chunk] + slot_in_chunk
    # idx value: half 0 -> row 1+src in t1a view; half 1 -> row src-25000
    # in t1b view (same values work for table2's shifted views)
    e_idx = np.where(e_h == 0, 1 + s_src, s_src - HALF_N).astype(np.int16)
    gflat[e_core, gtok] = e_idx

    g_h = grp_sect % 2
    g_core = grp_sect // 2
    stok = sbase[g_h, grp_chunk] + grp_rr
    sval = (g_h * AGG_H + (grp_dst - g_core * SHARD)).astype(np.int16)
    sflat[g_core, stok] = sval

    def wrap(a):
        n = a.shape[1]
        w = a.reshape(N_CORES, n // 16, 16).transpose(0, 2, 1)
        return np.ascontiguousarray(np.tile(w, (1, 8, 1)))

    return d_out, d_in, wrap(gflat), wrap(sflat), struct


_cache: dict = {}


def _run(inputs: dict, trace: bool = False, trace_cores=None):
    node_embeddings = np.asarray(inputs["node_embeddings"], np.float32)
    W1 = np.asarray(inputs["W1"], np.float32)
    b1 = np.asarray(inputs["b1"], np.float32)
    W2 = np.asarray(inputs["W2"], np.float32)
    b2 = np.asarray(inputs["b2"], np.float32)
    edge_index = np.asarray(inputs["edge_index"])

    d_out, d_in, gflat_w, sflat_w, struct = _preprocess(edge_index)

    key = repr(struct)
    if key not in _cache:
        _cache[key] = _build_program(struct)
    nc = _cache[key]

    x_pad = np.zeros((NPAD, F_IN), np.float32)
    x_pad[:N_NODES] = node_embeddings
    xT = np.ascontiguousarray(x_pad.T)
    W2p = np.zeros((HID, NCLS_PAD), np.float32)
    W2p[:, :NCLS] = W2
    b1bc = np.tile(b1[None, :], (128, 1)).astype(np.float32)
    b2p = np.zeros(NCLS_PAD, np.float32)
    b2p[:NCLS] = b2
    b2bc = np.tile(b2p[None, :], (128, 1)).astype(np.float32)
    ident = np.eye(128, dtype=np.float32)
    do_pad = np.zeros(NPAD, np.float32)
    do_pad[:N_NODES] = d_out
    dout_pm = np.ascontiguousarray(do_pad.reshape(NT_X, 128).T)

    in_maps = []
    for c in range(N_CORES):
        sh = slice(c * SHARD, (c + 1) * SHARD)
        dish_pad = np.zeros(NT_S * 128, np.float32)
        dish_pad[:SHARD] = d_in[sh]
        dosh_pad = np.zeros(NT_S * 128, np.float32)
        dosh_pad[:SHARD] = d_out[sh]
        in_maps.append({
            "xT": xT,
            "W1": W1,
            "W2p": W2p,
            "b1bc": b1bc,
            "b2bc": b2bc,
            "ident": ident,
            "dout_pm": dout_pm,
            "dinsh": np.ascontiguousarray(dish_pad.reshape(NT_S, 128).T),
            "doutsh": np.ascontiguousarray(dosh_pad.reshape(NT_S, 128).T),
            "gidx": gflat_w[c],
            "sidx": sflat_w[c],
        })

    kw = {}
    if trace:
        kw = dict(trace=True,
                  trace_cores=trace_cores if trace_cores else [0])
    res = bass_utils.run_bass_kernel_spmd(
        nc, in_maps, core_ids=list(range(N_CORES)), **kw)
    out = np.concatenate([r["out"] for r in res.results], axis=0)
    return out, res


def kernel(**inputs) -> np.ndarray:
    out, _ = _run(inputs, trace=False)
    return out
